# revision 7
# baseline (speedup 1.0000x reference)
"""MoE expert-parallel kernel for 8 TRN2 NeuronCores.

Problem: out[t] = sum_e w_e[t] * gelu(x[t] @ w1[e]) @ w2[e], top-2 routing,
8 experts == 8 cores. Strategy: expert parallelism with the dispatch/combine
("all-to-all") done on host — each core runs a dense FFN for exactly one
expert over the tokens routed to it (padded to a common capacity C), with
w1/w2 resident in SBUF as bf16 and all matmuls at bf16 rate with fp32
accumulation.
"""

import os
import sys
import types

import numpy as np
import ml_dtypes

from concourse import bacc, bass, mybir, tile
from concourse.bass_utils import run_bass_kernel_spmd


def _harden_trace_path():
    """If BASS_TRACE is set in the environment, run_bass_kernel_spmd imports
    antenv.axon_hooks, which is missing on this image; synthesize it from
    trn_agent_boot so tracing works instead of crashing. Also make the
    artifact upload degrade to a local path when no object store is
    reachable. Both are no-ops when the real modules work."""
    try:
        try:
            from antenv import axon_hooks  # noqa: F401
        except ImportError:
            import antenv
            from trn_agent_boot.trn_boot import _ntff_profile_via_ctypes
            m = types.ModuleType("antenv.axon_hooks")
            m._hook = _ntff_profile_via_ctypes("/opt/axon/libaxon_pjrt.so")
            m.get_axon_ntff_profile_hook = lambda: m._hook
            m.set_axon_ntff_profile_hook = lambda h: setattr(m, "_hook", h)
            sys.modules["antenv.axon_hooks"] = m
            antenv.axon_hooks = m
    except Exception:
        pass
    try:
        from concourse import bass_utils as _bu
        _orig_upload = _bu.upload_artifacts

        def _safe_upload(tmpdir):
            try:
                return _orig_upload(tmpdir)
            except Exception:
                return f"local:{tmpdir}"

        _bu.upload_artifacts = _safe_upload
    except Exception:
        pass


_harden_trace_path()

N_EXPERTS = 8
D_MODEL = 1024
D_FF = 4096
N_CORES = 8

BF16 = mybir.dt.bfloat16
F32 = mybir.dt.float32

# cache of compiled graphs keyed by (capacity, d_model, d_ff)
_GRAPH_CACHE = {}
LAST_RESULTS = None  # BassKernelResults of the most recent run (for test.py)


def _token_tiles(C):
    """Split capacity C (multiple of 128) into token tiles: 512s + remainder."""
    tiles = []
    off = 0
    while C - off >= 512:
        tiles.append((off, 512))
        off += 512
    if C - off > 0:
        tiles.append((off, C - off))
        off = C
    return tiles


def _build_graph(C, d_model=D_MODEL, d_ff=D_FF):
    """Build the per-core Bass graph for capacity C tokens.

    Inputs (per core): xT [d_model, C] bf16, w1 [d_model, d_ff] bf16,
    w2 [d_ff, d_model] bf16. Output: y [C, d_model] f32.
    """
    assert d_model % 512 == 0 and d_ff % 128 == 0 and C % 128 == 0
    nc = bacc.Bacc("TRN2", target_bir_lowering=False, debug=False,
                   num_devices=N_CORES)

    xT_d = nc.dram_tensor("xT", [d_model, C], BF16, kind="ExternalInput").ap()
    w1_d = nc.dram_tensor("w1", [d_model, d_ff], BF16, kind="ExternalInput").ap()
    w2_d = nc.dram_tensor("w2", [d_ff, d_model], BF16, kind="ExternalInput").ap()
    y_d = nc.dram_tensor("y", [C, d_model], F32, kind="ExternalOutput").ap()

    KD = d_model // 128   # k-chunks for matmul1
    KF = d_ff // 128      # dff-chunks
    ND = d_model // 512   # output column chunks

    tiles = _token_tiles(C)
    gelu = mybir.ActivationFunctionType.Gelu_apprx_tanh

    with tile.TileContext(nc) as tc:
        with (
            tc.tile_pool(name="weights", bufs=1) as wpool,
            tc.tile_pool(name="xin", bufs=2) as xpool,
            tc.tile_pool(name="hbuf", bufs=1) as hpool,
            tc.tile_pool(name="yout", bufs=4) as ypool,
            tc.tile_pool(name="ps1", bufs=4, space="PSUM") as ps1pool,
            tc.tile_pool(name="ps2", bufs=4, space="PSUM") as ps2pool,
        ):
            # --- DMA order matters: x tile 0 first, then w1 (k-ascending, so
            # tile-0 matmuls can start as chunks land), then x tile 1, then w2
            # (only needed for phase B, ~60us in). All on the sync queue so
            # order is strict and HBM bandwidth isn't split. y-out DMAs go on
            # gpsimd's queue.
            x_tiles_sb = {}

            def load_x(ti, t0, TT, eng=None):
                eng = eng or nc.sync
                x_sb = []
                for k in range(KD):
                    xt = xpool.tile([128, 512], BF16, name=f"xsb{k}", tag=f"xsb{k}")
                    eng.dma_start(out=xt[:, :TT],
                                  in_=xT_d[k * 128:(k + 1) * 128, t0:t0 + TT])
                    x_sb.append(xt)
                x_tiles_sb[ti] = x_sb

            load_x(0, tiles[0][0], tiles[0][1], eng=nc.gpsimd)

            w1_sb = []
            for k in range(KD):
                t = wpool.tile([128, d_ff], BF16, name=f"w1sb{k}", tag=f"w1sb{k}")
                nc.sync.dma_start(out=t[:], in_=w1_d[k * 128:(k + 1) * 128, :])
                w1_sb.append(t)

            if len(tiles) > 1:
                load_x(1, tiles[1][0], tiles[1][1])

            w2_sb = []
            for f in range(KF):
                t = wpool.tile([128, d_model], BF16, name=f"w2sb{f}", tag=f"w2sb{f}")
                nc.sync.dma_start(out=t[:], in_=w2_d[f * 128:(f + 1) * 128, :])
                w2_sb.append(t)

            # hT chunk buffers (shared across token tiles, single-buffered)
            h_sb = [
                hpool.tile([128, 512], BF16, name=f"hsb{f}", tag=f"hsb{f}")
                for f in range(KF)
            ]

            for ti, (t0, TT) in enumerate(tiles):
                if ti not in x_tiles_sb:
                    load_x(ti, t0, TT)
                x_sb = x_tiles_sb.pop(ti)

                # ---- matmul1 + gelu: hT[f] = gelu(w1[:,f].T @ xT) ----
                if ti == 0:
                    # k-outer over fc-groups of 4: consume w1 chunks as the
                    # DMA delivers them instead of stalling on the full w1.
                    for gi, g in enumerate(range(0, KF, 4)):
                        pool = ps1pool if gi % 2 == 0 else ps2pool
                        ptag = "ps1" if gi % 2 == 0 else "ps2"
                        pss = []
                        for f in range(g, g + 4):
                            ps1 = pool.tile([128, 512], F32, name="ps1",
                                            tag=ptag)
                            pss.append(ps1)
                        for k in range(KD):
                            for j, f in enumerate(range(g, g + 4)):
                                nc.tensor.matmul(
                                    pss[j][:, :TT],
                                    w1_sb[k][:, f * 128:(f + 1) * 128],
                                    x_sb[k][:, :TT],
                                    start=(k == 0),
                                    stop=(k == KD - 1),
                                )
                        for j, f in enumerate(range(g, g + 4)):
                            for c0 in range(0, TT, 128):
                                nc.scalar.activation(
                                    h_sb[f][:, c0:c0 + 128],
                                    pss[j][:, c0:c0 + 128], gelu)
                else:
                    for f in range(KF):
                        ps1 = ps1pool.tile([128, 512], F32, name="ps1", tag="ps1")
                        for k in range(KD):
                            nc.tensor.matmul(
                                ps1[:, :TT],
                                w1_sb[k][:, f * 128:(f + 1) * 128],
                                x_sb[k][:, :TT],
                                start=(k == 0),
                                stop=(k == KD - 1),
                            )
                        for c0 in range(0, TT, 128):
                            nc.scalar.activation(h_sb[f][:, c0:c0 + 128],
                                                 ps1[:, c0:c0 + 128], gelu)

                # ---- matmul2: y[ts, dc] = hT[:, ts].T @ w2[:, dc] ----
                for ts in range(TT // 128):
                    for dc in range(ND):
                        ps2 = ps2pool.tile([128, 512], F32, name="ps2", tag="ps2")
                        for f in range(KF):
                            nc.tensor.matmul(
                                ps2[:],
                                h_sb[f][:, ts * 128:(ts + 1) * 128],
                                w2_sb[f][:, dc * 512:(dc + 1) * 512],
                                start=(f == 0),
                                stop=(f == KF - 1),
                            )
                        ysb = ypool.tile([128, 512], F32, name="ysb", tag="ysb")
                        for c0 in range(0, 512, 128):
                            nc.vector.tensor_copy(ysb[:, c0:c0 + 128],
                                                  ps2[:, c0:c0 + 128])
                        nc.gpsimd.dma_start(
                            out=y_d[t0 + ts * 128:t0 + (ts + 1) * 128,
                                    dc * 512:(dc + 1) * 512],
                            in_=ysb[:],
                        )

    nc.compile()
    return nc


def kernel(hidden_states, selected_experts, routing_weights, w1, w2):
    global LAST_RESULTS

    hs = np.asarray(hidden_states, dtype=np.float32)
    sel = np.asarray(selected_experts)
    rw = np.asarray(routing_weights, dtype=np.float32)
    w1 = np.asarray(w1, dtype=np.float32)
    w2 = np.asarray(w2, dtype=np.float32)

    n_tokens, d_model = hs.shape
    top_k = sel.shape[1]
    n_experts, _, d_ff = w1.shape
    assert n_experts == N_CORES, "one expert per core"

    # ---- host dispatch: sort assignments by expert ----
    flat_e = np.ascontiguousarray(sel).reshape(-1).astype(np.int64)
    order = np.argsort(flat_e, kind="stable")          # assignment ids sorted by expert
    counts = np.bincount(flat_e, minlength=n_experts)
    starts = np.zeros(n_experts + 1, dtype=np.int64)
    np.cumsum(counts, out=starts[1:])
    token_of = order // top_k                          # token index per sorted assignment

    C = max(128 * int(np.ceil(counts.max() / 128)), 512)

    # per-core inputs
    w1_bf = w1.astype(ml_dtypes.bfloat16)
    w2_bf = w2.astype(ml_dtypes.bfloat16)
    in_maps = []
    for e in range(n_experts):
        toks = token_of[starts[e]:starts[e + 1]]
        xT = np.zeros((d_model, C), dtype=ml_dtypes.bfloat16)
        if len(toks):
            xT[:, :len(toks)] = hs[toks].T.astype(ml_dtypes.bfloat16)
        in_maps.append({"xT": xT, "w1": w1_bf[e], "w2": w2_bf[e]})

    key = (C, d_model, d_ff)
    nc = _GRAPH_CACHE.get(key)
    if nc is None:
        nc = _build_graph(C, d_model, d_ff)
        _GRAPH_CACHE[key] = nc

    res = run_bass_kernel_spmd(nc, in_maps, core_ids=list(range(N_CORES)))
    LAST_RESULTS = res

    # ---- host combine ----
    # res_sorted[p] = expert-FFN output row for sorted assignment p
    res_sorted = np.empty((n_tokens * top_k, d_model), dtype=np.float32)
    for e in range(n_experts):
        cnt = int(counts[e])
        if cnt:
            res_sorted[starts[e]:starts[e + 1]] = res.results[e]["y"][:cnt]

    inv = np.empty_like(order)
    inv[order] = np.arange(len(order))
    per_assign = res_sorted[inv].reshape(n_tokens, top_k, d_model)
    out = np.einsum("tkd,tk->td", per_assign, rw).astype(np.float32)
    return out


# revision 9
# speedup vs baseline: 1.0044x; 1.0044x over previous
"""MoE expert-parallel kernel for 8 TRN2 NeuronCores.

Problem: out[t] = sum_e w_e[t] * gelu(x[t] @ w1[e]) @ w2[e], top-2 routing,
8 experts == 8 cores. Strategy: expert parallelism with the dispatch/combine
("all-to-all") done on host — each core runs a dense FFN for exactly one
expert over the tokens routed to it (padded to a common capacity C), with
w1/w2 resident in SBUF as bf16 and all matmuls at bf16 rate with fp32
accumulation.
"""


import sys
import types

import numpy as np
import ml_dtypes

from concourse import bacc, bass, mybir, tile
from concourse.bass_utils import run_bass_kernel_spmd


def _harden_trace_path():
    """If BASS_TRACE is set in the environment, run_bass_kernel_spmd imports
    antenv.axon_hooks, which is missing on this image; synthesize it from
    trn_agent_boot so tracing works instead of crashing. Also make the
    artifact upload degrade to a local path when no object store is
    reachable. Both are no-ops when the real modules work."""
    try:
        try:
            from antenv import axon_hooks  # noqa: F401
        except ImportError:
            import antenv
            from trn_agent_boot.trn_boot import _ntff_profile_via_ctypes
            m = types.ModuleType("antenv.axon_hooks")
            m._hook = _ntff_profile_via_ctypes("/opt/axon/libaxon_pjrt.so")
            m.get_axon_ntff_profile_hook = lambda: m._hook
            m.set_axon_ntff_profile_hook = lambda h: setattr(m, "_hook", h)
            sys.modules["antenv.axon_hooks"] = m
            antenv.axon_hooks = m
    except Exception:
        pass
    try:
        from concourse import bass_utils as _bu
        _orig_upload = _bu.upload_artifacts

        def _safe_upload(tmpdir):
            try:
                return _orig_upload(tmpdir)
            except Exception:
                return f"local:{tmpdir}"

        _bu.upload_artifacts = _safe_upload
    except Exception:
        pass


_harden_trace_path()

N_EXPERTS = 8
D_MODEL = 1024
D_FF = 4096
N_CORES = 8

BF16 = mybir.dt.bfloat16
F32 = mybir.dt.float32

# cache of compiled graphs keyed by (capacity, d_model, d_ff)
_GRAPH_CACHE = {}
LAST_RESULTS = None  # BassKernelResults of the most recent run (for test.py)


def _token_tiles(C):
    """Split capacity C (multiple of 128) into token tiles: 512s + remainder."""
    tiles = []
    off = 0
    while C - off >= 512:
        tiles.append((off, 512))
        off += 512
    if C - off > 0:
        tiles.append((off, C - off))
        off = C
    return tiles


def _build_graph(C, d_model=D_MODEL, d_ff=D_FF):
    """Build the per-core Bass graph for capacity C tokens.

    Inputs (per core): xT [d_model, C] bf16, w1 [d_model, d_ff] bf16,
    w2 [d_ff, d_model] bf16. Output: y [C, d_model] f32.
    """
    assert d_model % 512 == 0 and d_ff % 128 == 0 and C % 128 == 0
    nc = bacc.Bacc("TRN2", target_bir_lowering=False, debug=False,
                   num_devices=N_CORES)

    xT_d = nc.dram_tensor("xT", [d_model, C], BF16, kind="ExternalInput").ap()
    w1_d = nc.dram_tensor("w1", [d_model, d_ff], BF16, kind="ExternalInput").ap()
    w2_d = nc.dram_tensor("w2", [d_ff, d_model], BF16, kind="ExternalInput").ap()
    y_d = nc.dram_tensor("y", [C, d_model], F32, kind="ExternalOutput").ap()

    KD = d_model // 128   # k-chunks for matmul1
    KF = d_ff // 128      # dff-chunks
    ND = d_model // 512   # output column chunks

    tiles = _token_tiles(C)
    gelu = mybir.ActivationFunctionType.Gelu_apprx_tanh

    with tile.TileContext(nc) as tc:
        with (
            tc.tile_pool(name="weights", bufs=1) as wpool,
            tc.tile_pool(name="xin", bufs=2) as xpool,
            tc.tile_pool(name="hbuf", bufs=1) as hpool,
            tc.tile_pool(name="yout", bufs=4) as ypool,
            tc.tile_pool(name="ps1", bufs=4, space="PSUM") as ps1pool,
            tc.tile_pool(name="ps2", bufs=4, space="PSUM") as ps2pool,
        ):
            # --- DMA order matters: x tile 0 first, then w1 (k-ascending, so
            # tile-0 matmuls can start as chunks land), then x tile 1, then w2
            # (only needed for phase B, ~60us in). All on the sync queue so
            # order is strict and HBM bandwidth isn't split. y-out DMAs go on
            # gpsimd's queue.
            x_tiles_sb = {}

            def load_x(ti, t0, TT, eng=None):
                eng = eng or nc.sync
                x_sb = []
                for k in range(KD):
                    xt = xpool.tile([128, 512], BF16, name=f"xsb{k}", tag=f"xsb{k}")
                    eng.dma_start(out=xt[:, :TT],
                                  in_=xT_d[k * 128:(k + 1) * 128, t0:t0 + TT])
                    x_sb.append(xt)
                x_tiles_sb[ti] = x_sb

            load_x(0, tiles[0][0], tiles[0][1], eng=nc.gpsimd)

            # w1 DMAs issue fc-group-major: the first ~1MB delivers all 8
            # k-chunks of fc-group 0, so tile-0's first accumulation chains
            # can close immediately instead of waiting for the full w1
            # (subtile deps let each matmul wait only on the slice it reads).
            w1_sb = [
                wpool.tile([128, d_ff], BF16, name=f"w1sb{k}", tag=f"w1sb{k}")
                for k in range(KD)
            ]
            for g0 in range(0, d_ff, 512):
                for k in range(KD):
                    nc.sync.dma_start(
                        out=w1_sb[k][:, g0:g0 + 512],
                        in_=w1_d[k * 128:(k + 1) * 128, g0:g0 + 512])

            if len(tiles) > 1:
                load_x(1, tiles[1][0], tiles[1][1])

            w2_sb = []
            for f in range(KF):
                t = wpool.tile([128, d_model], BF16, name=f"w2sb{f}", tag=f"w2sb{f}")
                nc.sync.dma_start(out=t[:], in_=w2_d[f * 128:(f + 1) * 128, :])
                w2_sb.append(t)

            # hT chunk buffers (shared across token tiles, single-buffered)
            h_sb = [
                hpool.tile([128, 512], BF16, name=f"hsb{f}", tag=f"hsb{f}")
                for f in range(KF)
            ]

            for ti, (t0, TT) in enumerate(tiles):
                if ti not in x_tiles_sb:
                    load_x(ti, t0, TT)
                x_sb = x_tiles_sb.pop(ti)

                # ---- matmul1 + gelu: hT[f] = gelu(w1[:,f].T @ xT) ----
                if ti == 0:
                    # k-outer over fc-groups of 4: consume w1 chunks as the
                    # DMA delivers them instead of stalling on the full w1.
                    for gi, g in enumerate(range(0, KF, 4)):
                        pool = ps1pool if gi % 2 == 0 else ps2pool
                        ptag = "ps1" if gi % 2 == 0 else "ps2"
                        pss = []
                        for f in range(g, g + 4):
                            ps1 = pool.tile([128, 512], F32, name="ps1",
                                            tag=ptag)
                            pss.append(ps1)
                        for k in range(KD):
                            for j, f in enumerate(range(g, g + 4)):
                                nc.tensor.matmul(
                                    pss[j][:, :TT],
                                    w1_sb[k][:, f * 128:(f + 1) * 128],
                                    x_sb[k][:, :TT],
                                    start=(k == 0),
                                    stop=(k == KD - 1),
                                )
                        for j, f in enumerate(range(g, g + 4)):
                            for c0 in range(0, TT, 128):
                                nc.scalar.activation(
                                    h_sb[f][:, c0:c0 + 128],
                                    pss[j][:, c0:c0 + 128], gelu)
                else:
                    for f in range(KF):
                        ps1 = ps1pool.tile([128, 512], F32, name="ps1", tag="ps1")
                        for k in range(KD):
                            nc.tensor.matmul(
                                ps1[:, :TT],
                                w1_sb[k][:, f * 128:(f + 1) * 128],
                                x_sb[k][:, :TT],
                                start=(k == 0),
                                stop=(k == KD - 1),
                            )
                        for c0 in range(0, TT, 128):
                            nc.scalar.activation(h_sb[f][:, c0:c0 + 128],
                                                 ps1[:, c0:c0 + 128], gelu)

                # ---- matmul2: y[ts, dc] = hT[:, ts].T @ w2[:, dc] ----
                for ts in range(TT // 128):
                    for dc in range(ND):
                        ps2 = ps2pool.tile([128, 512], F32, name="ps2", tag="ps2")
                        for f in range(KF):
                            nc.tensor.matmul(
                                ps2[:],
                                h_sb[f][:, ts * 128:(ts + 1) * 128],
                                w2_sb[f][:, dc * 512:(dc + 1) * 512],
                                start=(f == 0),
                                stop=(f == KF - 1),
                            )
                        ysb = ypool.tile([128, 512], F32, name="ysb", tag="ysb")
                        for c0 in range(0, 512, 128):
                            nc.vector.tensor_copy(ysb[:, c0:c0 + 128],
                                                  ps2[:, c0:c0 + 128])
                        nc.gpsimd.dma_start(
                            out=y_d[t0 + ts * 128:t0 + (ts + 1) * 128,
                                    dc * 512:(dc + 1) * 512],
                            in_=ysb[:],
                        )

    nc.compile()
    return nc


def kernel(hidden_states, selected_experts, routing_weights, w1, w2):
    global LAST_RESULTS

    hs = np.asarray(hidden_states, dtype=np.float32)
    sel = np.asarray(selected_experts)
    rw = np.asarray(routing_weights, dtype=np.float32)
    w1 = np.asarray(w1, dtype=np.float32)
    w2 = np.asarray(w2, dtype=np.float32)

    n_tokens, d_model = hs.shape
    top_k = sel.shape[1]
    n_experts, _, d_ff = w1.shape
    assert n_experts == N_CORES, "one expert per core"

    # ---- host dispatch: sort assignments by expert ----
    flat_e = np.ascontiguousarray(sel).reshape(-1).astype(np.int64)
    order = np.argsort(flat_e, kind="stable")          # assignment ids sorted by expert
    counts = np.bincount(flat_e, minlength=n_experts)
    starts = np.zeros(n_experts + 1, dtype=np.int64)
    np.cumsum(counts, out=starts[1:])
    token_of = order // top_k                          # token index per sorted assignment

    C = max(128 * int(np.ceil(counts.max() / 128)), 512)

    # per-core inputs
    w1_bf = w1.astype(ml_dtypes.bfloat16)
    w2_bf = w2.astype(ml_dtypes.bfloat16)
    in_maps = []
    for e in range(n_experts):
        toks = token_of[starts[e]:starts[e + 1]]
        xT = np.zeros((d_model, C), dtype=ml_dtypes.bfloat16)
        if len(toks):
            xT[:, :len(toks)] = hs[toks].T.astype(ml_dtypes.bfloat16)
        in_maps.append({"xT": xT, "w1": w1_bf[e], "w2": w2_bf[e]})

    key = (C, d_model, d_ff)
    nc = _GRAPH_CACHE.get(key)
    if nc is None:
        nc = _build_graph(C, d_model, d_ff)
        _GRAPH_CACHE[key] = nc

    res = run_bass_kernel_spmd(nc, in_maps, core_ids=list(range(N_CORES)))
    LAST_RESULTS = res

    # ---- host combine ----
    # res_sorted[p] = expert-FFN output row for sorted assignment p
    res_sorted = np.empty((n_tokens * top_k, d_model), dtype=np.float32)
    for e in range(n_experts):
        cnt = int(counts[e])
        if cnt:
            res_sorted[starts[e]:starts[e + 1]] = res.results[e]["y"][:cnt]

    inv = np.empty_like(order)
    inv[order] = np.arange(len(order))
    per_assign = res_sorted[inv].reshape(n_tokens, top_k, d_model)
    out = np.einsum("tkd,tk->td", per_assign, rw).astype(np.float32)
    return out


# revision 10
# speedup vs baseline: 1.0121x; 1.0077x over previous
"""MoE expert-parallel kernel for 8 TRN2 NeuronCores.

Problem: out[t] = sum_e w_e[t] * gelu(x[t] @ w1[e]) @ w2[e], top-2 routing,
8 experts == 8 cores. Strategy: expert parallelism with the dispatch/combine
("all-to-all") done on host — each core runs a dense FFN for exactly one
expert over the tokens routed to it (padded to a common capacity C), with
w1/w2 resident in SBUF as bf16 and all matmuls at bf16 rate with fp32
accumulation.
"""


import sys
import types

import numpy as np
import ml_dtypes

from concourse import bacc, bass, mybir, tile
from concourse.bass_utils import run_bass_kernel_spmd


def _harden_trace_path():
    """If BASS_TRACE is set in the environment, run_bass_kernel_spmd imports
    antenv.axon_hooks, which is missing on this image; synthesize it from
    trn_agent_boot so tracing works instead of crashing. Also make the
    artifact upload degrade to a local path when no object store is
    reachable. Both are no-ops when the real modules work."""
    try:
        try:
            from antenv import axon_hooks  # noqa: F401
        except ImportError:
            import antenv
            from trn_agent_boot.trn_boot import _ntff_profile_via_ctypes
            m = types.ModuleType("antenv.axon_hooks")
            m._hook = _ntff_profile_via_ctypes("/opt/axon/libaxon_pjrt.so")
            m.get_axon_ntff_profile_hook = lambda: m._hook
            m.set_axon_ntff_profile_hook = lambda h: setattr(m, "_hook", h)
            sys.modules["antenv.axon_hooks"] = m
            antenv.axon_hooks = m
    except Exception:
        pass
    try:
        from concourse import bass_utils as _bu
        _orig_upload = _bu.upload_artifacts

        def _safe_upload(tmpdir):
            try:
                return _orig_upload(tmpdir)
            except Exception:
                return f"local:{tmpdir}"

        _bu.upload_artifacts = _safe_upload
    except Exception:
        pass


_harden_trace_path()

N_EXPERTS = 8
D_MODEL = 1024
D_FF = 4096
N_CORES = 8

BF16 = mybir.dt.bfloat16
F32 = mybir.dt.float32

# cache of compiled graphs keyed by (capacity, d_model, d_ff)
_GRAPH_CACHE = {}
LAST_RESULTS = None  # BassKernelResults of the most recent run (for test.py)


def _token_tiles(C):
    """Split capacity C (multiple of 128) into token tiles: 512s + remainder."""
    tiles = []
    off = 0
    while C - off >= 512:
        tiles.append((off, 512))
        off += 512
    if C - off > 0:
        tiles.append((off, C - off))
        off = C
    return tiles


def _build_graph(C, d_model=D_MODEL, d_ff=D_FF):
    """Build the per-core Bass graph for capacity C tokens.

    Inputs (per core): xT [d_model, C] bf16, w1 [d_model, d_ff] bf16,
    w2 [d_ff, d_model] bf16. Output: y [C, d_model] f32.
    """
    assert d_model % 512 == 0 and d_ff % 128 == 0 and C % 128 == 0
    nc = bacc.Bacc("TRN2", target_bir_lowering=False, debug=False,
                   num_devices=N_CORES)

    KD = d_model // 128   # k-chunks for matmul1
    KF = d_ff // 128      # dff-chunks
    ND = d_model // 512   # output column chunks

    # Inputs use a partition-major interleave ([128, chunk, cols]) so a
    # single DMA can deliver one column-group across ALL contraction chunks
    # (each dma_start costs ~650ns of queue issue time; fewer, wider DMAs
    # keep the startup issue-bound path short while preserving
    # fc-group-major delivery order for w1).
    xT_d = nc.dram_tensor("xT", [128, KD, C], BF16, kind="ExternalInput").ap()
    w1_d = nc.dram_tensor("w1", [128, KD, d_ff], BF16, kind="ExternalInput").ap()
    w2_d = nc.dram_tensor("w2", [128, KF, d_model], BF16, kind="ExternalInput").ap()
    y_d = nc.dram_tensor("y", [C, d_model], F32, kind="ExternalOutput").ap()

    tiles = _token_tiles(C)
    gelu = mybir.ActivationFunctionType.Gelu_apprx_tanh

    with tile.TileContext(nc) as tc:
        with (
            tc.tile_pool(name="weights", bufs=1) as wpool,
            tc.tile_pool(name="xin", bufs=2) as xpool,
            tc.tile_pool(name="hbuf", bufs=1) as hpool,
            tc.tile_pool(name="yout", bufs=4) as ypool,
            tc.tile_pool(name="ps1", bufs=4, space="PSUM") as ps1pool,
            tc.tile_pool(name="ps2", bufs=4, space="PSUM") as ps2pool,
        ):
            # --- DMA order matters: x tile 0 first, then w1 (k-ascending, so
            # tile-0 matmuls can start as chunks land), then x tile 1, then w2
            # (only needed for phase B, ~60us in). All on the sync queue so
            # order is strict and HBM bandwidth isn't split. y-out DMAs go on
            # gpsimd's queue.
            x_tiles_sb = {}

            def load_x(ti, t0, TT, eng=None):
                eng = eng or nc.sync
                xt = xpool.tile([128, KD, 512], BF16, name="xsb", tag="xsb")
                eng.dma_start(out=xt[:, :, :TT], in_=xT_d[:, :, t0:t0 + TT])
                x_tiles_sb[ti] = xt

            load_x(0, tiles[0][0], tiles[0][1], eng=nc.gpsimd)

            # w1 arrives fc-group-major: each DMA delivers one 512-col group
            # across all KD k-chunks, so tile-0's accumulation chains close as
            # soon as ~1MB lands (subtile deps gate each matmul only on the
            # slice it reads).
            w1_all = wpool.tile([128, KD, d_ff], BF16, name="w1sb", tag="w1sb")
            for g0 in range(0, d_ff, 512):
                nc.sync.dma_start(out=w1_all[:, :, g0:g0 + 512],
                                  in_=w1_d[:, :, g0:g0 + 512])

            if len(tiles) > 1:
                load_x(1, tiles[1][0], tiles[1][1])

            w2_all = wpool.tile([128, KF, d_model], BF16, name="w2sb", tag="w2sb")
            for q in range(0, KF, 8):
                nc.sync.dma_start(out=w2_all[:, q:q + 8, :],
                                  in_=w2_d[:, q:q + 8, :])

            # hT chunk buffers (shared across token tiles, single-buffered)
            h_sb = [
                hpool.tile([128, 512], BF16, name=f"hsb{f}", tag=f"hsb{f}")
                for f in range(KF)
            ]

            for ti, (t0, TT) in enumerate(tiles):
                if ti not in x_tiles_sb:
                    load_x(ti, t0, TT)
                x_all = x_tiles_sb.pop(ti)

                # ---- matmul1 + gelu: hT[f] = gelu(w1[:,f].T @ xT) ----
                if ti == 0:
                    # k-outer over fc-groups of 4: consume w1 chunks as the
                    # DMA delivers them instead of stalling on the full w1.
                    for gi, g in enumerate(range(0, KF, 4)):
                        pool = ps1pool if gi % 2 == 0 else ps2pool
                        ptag = "ps1" if gi % 2 == 0 else "ps2"
                        pss = []
                        for f in range(g, g + 4):
                            ps1 = pool.tile([128, 512], F32, name="ps1",
                                            tag=ptag)
                            pss.append(ps1)
                        for k in range(KD):
                            for j, f in enumerate(range(g, g + 4)):
                                nc.tensor.matmul(
                                    pss[j][:, :TT],
                                    w1_all[:, k, f * 128:(f + 1) * 128],
                                    x_all[:, k, :TT],
                                    start=(k == 0),
                                    stop=(k == KD - 1),
                                )
                        for j, f in enumerate(range(g, g + 4)):
                            for c0 in range(0, TT, 128):
                                nc.scalar.activation(
                                    h_sb[f][:, c0:c0 + 128],
                                    pss[j][:, c0:c0 + 128], gelu)
                else:
                    for f in range(KF):
                        ps1 = ps1pool.tile([128, 512], F32, name="ps1", tag="ps1")
                        for k in range(KD):
                            nc.tensor.matmul(
                                ps1[:, :TT],
                                w1_all[:, k, f * 128:(f + 1) * 128],
                                x_all[:, k, :TT],
                                start=(k == 0),
                                stop=(k == KD - 1),
                            )
                        for c0 in range(0, TT, 128):
                            nc.scalar.activation(h_sb[f][:, c0:c0 + 128],
                                                 ps1[:, c0:c0 + 128], gelu)

                # ---- matmul2: y[ts, dc] = hT[:, ts].T @ w2[:, dc] ----
                for ts in range(TT // 128):
                    for dc in range(ND):
                        ps2 = ps2pool.tile([128, 512], F32, name="ps2", tag="ps2")
                        for f in range(KF):
                            nc.tensor.matmul(
                                ps2[:],
                                h_sb[f][:, ts * 128:(ts + 1) * 128],
                                w2_all[:, f, dc * 512:(dc + 1) * 512],
                                start=(f == 0),
                                stop=(f == KF - 1),
                            )
                        ysb = ypool.tile([128, 512], F32, name="ysb", tag="ysb")
                        for c0 in range(0, 512, 128):
                            nc.vector.tensor_copy(ysb[:, c0:c0 + 128],
                                                  ps2[:, c0:c0 + 128])
                        nc.gpsimd.dma_start(
                            out=y_d[t0 + ts * 128:t0 + (ts + 1) * 128,
                                    dc * 512:(dc + 1) * 512],
                            in_=ysb[:],
                        )

    nc.compile()
    return nc


def kernel(hidden_states, selected_experts, routing_weights, w1, w2):
    global LAST_RESULTS

    hs = np.asarray(hidden_states, dtype=np.float32)
    sel = np.asarray(selected_experts)
    rw = np.asarray(routing_weights, dtype=np.float32)
    w1 = np.asarray(w1, dtype=np.float32)
    w2 = np.asarray(w2, dtype=np.float32)

    n_tokens, d_model = hs.shape
    top_k = sel.shape[1]
    n_experts, _, d_ff = w1.shape
    assert n_experts == N_CORES, "one expert per core"

    # ---- host dispatch: sort assignments by expert ----
    flat_e = np.ascontiguousarray(sel).reshape(-1).astype(np.int64)
    order = np.argsort(flat_e, kind="stable")          # assignment ids sorted by expert
    counts = np.bincount(flat_e, minlength=n_experts)
    starts = np.zeros(n_experts + 1, dtype=np.int64)
    np.cumsum(counts, out=starts[1:])
    token_of = order // top_k                          # token index per sorted assignment

    C = max(128 * int(np.ceil(counts.max() / 128)), 512)

    # per-core inputs, partition-major interleave ([128, chunk, cols])
    KD = d_model // 128
    KF = d_ff // 128
    w1_bf = w1.astype(ml_dtypes.bfloat16)
    w2_bf = w2.astype(ml_dtypes.bfloat16)
    in_maps = []
    for e in range(n_experts):
        toks = token_of[starts[e]:starts[e + 1]]
        xT = np.zeros((128, KD, C), dtype=ml_dtypes.bfloat16)
        if len(toks):
            xt = hs[toks].astype(ml_dtypes.bfloat16)  # [cnt, d_model]
            xT[:, :, :len(toks)] = xt.T.reshape(KD, 128, len(toks)).transpose(1, 0, 2)
        w1i = np.ascontiguousarray(
            w1_bf[e].reshape(KD, 128, d_ff).transpose(1, 0, 2))
        w2i = np.ascontiguousarray(
            w2_bf[e].reshape(KF, 128, d_model).transpose(1, 0, 2))
        in_maps.append({"xT": xT, "w1": w1i, "w2": w2i})

    key = (C, d_model, d_ff)
    nc = _GRAPH_CACHE.get(key)
    if nc is None:
        nc = _build_graph(C, d_model, d_ff)
        _GRAPH_CACHE[key] = nc

    res = run_bass_kernel_spmd(nc, in_maps, core_ids=list(range(N_CORES)))
    LAST_RESULTS = res

    # ---- host combine ----
    # res_sorted[p] = expert-FFN output row for sorted assignment p
    res_sorted = np.empty((n_tokens * top_k, d_model), dtype=np.float32)
    for e in range(n_experts):
        cnt = int(counts[e])
        if cnt:
            res_sorted[starts[e]:starts[e + 1]] = res.results[e]["y"][:cnt]

    inv = np.empty_like(order)
    inv[order] = np.arange(len(order))
    per_assign = res_sorted[inv].reshape(n_tokens, top_k, d_model)
    out = np.einsum("tkd,tk->td", per_assign, rw).astype(np.float32)
    return out
